# revision 25
# baseline (speedup 1.0000x reference)
"""Trainium2 Bass kernel for BatchUnProcessDatav2.

Sharding: pure data-parallel over batch B=64 -> 8 samples per NeuronCore.

Math per (b, t) with root quaternion q=(w,x,y,z), inv = conj(q):
  rot_out = standardize(quat_mul(inv, glb_rot))
  pos_rot = R(inv) @ glb_pos        (and vel_rot for joint-0 velocity)
  traj[t] = pos_rot[0, j0] + cumsum(vel_rot_j0)[t-1]
  pos_out = pos_rot + traj * [1,0,1]

Layout: partition = 128 consecutive t of one sample ("bt tile"), free dim =
interleaved (slot, comp). pos tile V has 34 slots (32 joints, vel j0, pad).

quat_mul(inv, r) is computed as 4 full-tile products P_k = q_k * r followed
by a chain of 6 half-plane add/sub ops (signs+perms verified vs reference).
quat_apply uses v' = A*v + 2u(u.v) - 2w*(u x v), A = 2w^2-1, u = (x,y,z).

The time-scan runs on-chip: per tile the rotated j0 velocity [128,3] is
staged into a [128,24] buffer (8 samples), PE-transposed to [24,128] PSUM,
cumsum'd with tensor_tensor_scan ([24,1024] in 2 chunks), PE-transposed
back to [128,24] per t-block, and added to the held pos tiles in phase B.
"""

import numpy as np

import bass_rust
import concourse.bacc as bacc
import concourse.bass as bass
import concourse.mybir as mybir
from concourse import masks
from concourse.tile import TileContext

F = mybir.dt.float32
ALU = mybir.AluOpType
AX = mybir.AxisListType

B, T, J = 64, 1024, 32
NCORES = 8
BPC = B // NCORES          # samples per core
NTB = T // 128             # t-blocks per sample
P = 128

# engine assignment table (tunable): 'vector' | 'scalar' | 'gpsimd'
ENG = {
    "scalars": "gpsimd",
    "rp_w": "scalar", "rp_x": "scalar", "rp_y": "scalar", "rp_z": "gpsimd",
    "rcomb": "vector", "rcomb2": "vector",
    "std": "vector", "std_apply": "gpsimd",
    "p3": "vector",
    "dred": "vector",
    "s": "gpsimd",
    "asm": "vector",
    "stage": "vector",
    "phaseb": "gpsimd",
}


def _e(nc, key):
    return getattr(nc, ENG[key])


def _ts_mul(nc, key, out, in_, scalar):
    eng = ENG[key]
    if eng == "scalar":
        nc.scalar.mul(out, in_, scalar)
    else:
        getattr(nc, eng).tensor_scalar_mul(out, in_, scalar)


def _ap(base, offset_extra, dims):
    """Raw free-dim AP on the same tensor/partitions as `base` ([P, ...] AP)."""
    return bass_rust.AP(
        tensor=base.tensor,
        offset=base.offset + offset_extra,
        ap=[list(base.ap[0])] + [list(d) for d in dims],
    )


def build_nc():
    nc = bacc.Bacc("TRN2", target_bir_lowering=False, debug=False, num_devices=NCORES)
    g_posv = nc.dram_tensor("g_posv", [BPC, T, J + 2, 3], F, kind="ExternalInput")  # joints + j0 vel + pad
    g_rot = nc.dram_tensor("g_rot", [BPC, T, J, 4], F, kind="ExternalInput")
    g_root = nc.dram_tensor("g_root", [BPC, T, 4], F, kind="ExternalInput")
    o_pos = nc.dram_tensor("o_pos", [BPC, T, J, 3], F, kind="ExternalOutput")
    o_rot = nc.dram_tensor("o_rot", [BPC, T, J, 4], F, kind="ExternalOutput")

    with TileContext(nc) as tc:
        with (
            tc.tile_pool(name="io", bufs=6) as io,
            tc.tile_pool(name="prod", bufs=4) as prod,
            tc.tile_pool(name="work", bufs=4) as work,
            tc.tile_pool(name="keep", bufs=1) as keep,
            tc.tile_pool(name="opool", bufs=24) as opool,
            tc.tile_pool(name="qs", bufs=3) as qsp,
            tc.tile_pool(name="cumb", bufs=2) as cumbp,
            tc.tile_pool(name="ps", space="PSUM", bufs=1) as psp,
            tc.tile_pool(name="ps2", space="PSUM", bufs=2) as psp2,
            tc.tile_pool(name="ps3", space="PSUM", bufs=2) as psp3,
        ):
            ident = keep.tile([P, P], F)
            masks.make_identity(nc, ident[:])
            zer3 = keep.tile([3, 512], F)
            nc.vector.memset(zer3[:], 0.0)

            o_tiles = {}
            o_pairs = {}
            rot_elems = T * J * 4
            pos_elems = T * J * 3

            # ---------------- per-sample-pair pipeline ----------------
            for bp in range(BPC // 2):
                b0 = 2 * bp
                Qs = qsp.tile([P, 8 * NTB], F, tag="Qs")
                nc.sync.dma_start(
                    out=Qs[:],
                    in_=bass_rust.AP(
                        tensor=g_root[:].tensor, offset=b0 * T * 4,
                        ap=[[4, P], [T * 4, 2], [512, NTB], [1, 4]]),
                )
                trs = {
                    (0, 0): psp.tile([3, 512], F, tag="trA0", name="trA0"),
                    (0, 1): psp.tile([3, 512], F, tag="trA1", name="trA1"),
                    (1, 0): psp.tile([3, 512], F, tag="trB0", name="trB0"),
                    (1, 1): psp.tile([3, 512], F, tag="trB1", name="trB1"),
                }
                p0s = {h: psp2.tile([3, P], F, tag="p0ps", name="p0ps") for h in (0, 1)}

                for tb in range(NTB):
                    sl = slice(tb * 128, tb * 128 + 128)
                    Rp = io.tile([P, 256], F, tag="R")
                    Vp = io.tile([P, 204], F, tag="V")
                    nc.sync.dma_start(
                        out=Rp[:],
                        in_=bass_rust.AP(
                            tensor=g_rot[:].tensor,
                            offset=b0 * rot_elems + tb * 128 * J * 4,
                            ap=[[J * 4, P], [rot_elems, 2], [1, J * 4]]))
                    nc.sync.dma_start(
                        out=Vp[:],
                        in_=bass_rust.AP(
                            tensor=g_posv[:].tensor,
                            offset=b0 * T * 102 + tb * 128 * 102,
                            ap=[[102, P], [T * 102, 2], [1, 102]]))
                    ROTp = prod.tile([P, 256], F, tag="ROT")
                    Opair = opool.tile([P, 204], F, tag="O", name=f"O_{bp}_{tb}")
                    o_pairs[(bp, tb)] = Opair

                    for h in (0, 1):
                        b = b0 + h
                        Q = Qs[:, 32 * h + 4 * tb: 32 * h + 4 * tb + 4]
                        R = Rp[:, 128 * h: 128 * h + 128]
                        V = Vp[:, 102 * h: 102 * h + 102]
                        tr0, tr1 = trs[(h, 0)], trs[(h, 1)]
                        p0ps = p0s[h]

                        # --- per-partition scalars
                        se = _e(nc, "scalars")
                        T1 = work.tile([P, 4], F, tag="T1")
                        D2 = work.tile([P, 4], F, tag="D2")
                        A1 = work.tile([P, 1], F, tag="A1")
                        M2W = work.tile([P, 1], F, tag="M2W")
                        se.tensor_scalar_mul(T1[:], Q, Q[:, 0:1])
                        se.tensor_scalar_mul(D2[:], Q, 2.0)
                        se.tensor_scalar(A1[:], T1[:, 0:1], 2.0, 1.0, ALU.mult, ALU.subtract)
                        se.tensor_scalar_mul(M2W[:], Q[:, 0:1], -2.0)

                        # --- rot products (full contiguous tiles)
                        Pw = prod.tile([P, 128], F, tag="Pw")
                        Px = prod.tile([P, 128], F, tag="Px")
                        Py = prod.tile([P, 128], F, tag="Py")
                        Pz = prod.tile([P, 128], F, tag="Pz")
                        _ts_mul(nc, "rp_w", Pw[:], R, Q[:, 0:1])
                        _ts_mul(nc, "rp_x", Px[:], R, Q[:, 1:2])
                        _ts_mul(nc, "rp_y", Py[:], R, Q[:, 2:3])
                        _ts_mul(nc, "rp_z", Pz[:], R, Q[:, 3:4])

                        # --- rot combine: ACC planes via half add/sub chain
                        ce = _e(nc, "rcomb")
                        ACC = prod.tile([P, 128], F, tag="ACC")
                        J4 = [4, J]

                        def pl(t, offs, step):
                            return _ap(t[:], offs, [[step, 2], J4])

                        ce.tensor_tensor(out=pl(ACC, 0, 2), in0=pl(Pw, 0, 2), in1=pl(Px, 1, 2), op=ALU.add)
                        ce.tensor_tensor(out=pl(ACC, 1, 2), in0=pl(Pw, 1, 2), in1=pl(Px, 0, 2), op=ALU.subtract)
                        ce.tensor_tensor(out=pl(ACC, 0, 3), in0=pl(ACC, 0, 3), in1=pl(Py, 2, -1), op=ALU.add)
                        ce.tensor_tensor(out=pl(ACC, 1, 1), in0=pl(ACC, 1, 1), in1=pl(Py, 3, -3), op=ALU.subtract)
                        ce2 = _e(nc, "rcomb2")
                        ce2.tensor_tensor(out=pl(ACC, 0, 1), in0=pl(ACC, 0, 1), in1=pl(Pz, 3, -1), op=ALU.add)
                        ce2.tensor_tensor(out=pl(ACC, 2, 1), in0=pl(ACC, 2, 1), in1=pl(Pz, 1, -1), op=ALU.subtract)

                        # --- standardize into the pair rot tile
                        sde = _e(nc, "std")
                        CM = work.tile([P, J], F, tag="CM")
                        M = work.tile([P, J], F, tag="M")
                        wpl = _ap(ACC[:], 0, [[4, J]])
                        sde.tensor_scalar(CM[:], wpl, 0.0, None, ALU.is_lt)
                        sde.tensor_scalar(M[:], CM[:], -2.0, 1.0, ALU.mult, ALU.add)
                        _e(nc, "std_apply").tensor_tensor(
                            out=ROTp[:, 128 * h:128 * h + 128].rearrange("p (j c) -> p j c", c=4),
                            in0=ACC[:].rearrange("p (j c) -> p j c", c=4),
                            in1=M[:].unsqueeze(2).broadcast_to([P, J, 4]),
                            op=ALU.mult,
                        )

                        # --- pos: P3 = u_c * V
                        P3 = prod.tile([P, 306], F, tag="P3")
                        for c in range(3):
                            _ts_mul(nc, "p3", P3[:, 102 * c:102 * c + 102], V, Q[:, 1 + c:2 + c])

                        # d = u.v  (diagonal reduce over [[3,34],[103,3]])
                        D = work.tile([P, 34], F, tag="D")
                        _e(nc, "dred").tensor_reduce(
                            D[:], _ap(P3[:], 0, [[3, 34], [103, 3]]), axis=AX.X, op=ALU.add)

                        # s = u x v (per output plane)
                        S = work.tile([P, 102], F, tag="S")
                        se_ = _e(nc, "s")
                        for c, (ta, pa, tb2, pb) in enumerate([(1, 2, 2, 1), (2, 0, 0, 2), (0, 1, 1, 0)]):
                            se_.tensor_tensor(
                                out=_ap(S[:], c, [[3, 34]]),
                                in0=_ap(P3[:], 102 * ta + pa, [[3, 34]]),
                                in1=_ap(P3[:], 102 * tb2 + pb, [[3, 34]]),
                                op=ALU.subtract,
                            )

                        # O = A*V - 2w*S + 2u_c*d  (into pair tile half)
                        ae = _e(nc, "asm")
                        O = Opair[:, 102 * h:102 * h + 102]
                        ae.tensor_scalar_mul(O, V, A1[:])
                        ae.scalar_tensor_tensor(out=O, in0=S[:], scalar=M2W[:], in1=O, op0=ALU.mult, op1=ALU.add)
                        for c in range(3):
                            ae.scalar_tensor_tensor(
                                out=_ap(O, c, [[3, 34]]),
                                in0=D[:],
                                scalar=D2[:, 1 + c:2 + c],
                                in1=_ap(O, c, [[3, 34]]),
                                op0=ALU.mult, op1=ALU.add,
                            )
                        o_tiles[(b, tb)] = O

                        # --- vel (and t=0 joint-0 pos) transposes for the scan
                        trt = tr0 if tb < 4 else tr1
                        nc.tensor.transpose(
                            trt[:, (tb % 4) * 128:(tb % 4) * 128 + 128],
                            O[:, 96:99], ident[:])
                        if tb == 0:
                            nc.tensor.transpose(p0ps[:], O[:, 0:3], ident[:])

                    # pair rot-out DMA
                    nc.scalar.dma_start(
                        out=bass_rust.AP(
                            tensor=o_rot[:].tensor,
                            offset=b0 * rot_elems + tb * 128 * J * 4,
                            ap=[[J * 4, P], [rot_elems, 2], [1, J * 4]]),
                        in_=ROTp[:])

                # ---------------- per-sample scans ----------------
                cumbs = {}
                for h in (0, 1):
                    cumb = cumbp.tile([3, 1 + T], F, tag="cumb")
                    nc.vector.tensor_copy(cumb[:, 0:1], p0s[h][:, 0:1])
                    nc.vector.tensor_tensor_scan(
                        cumb[:, 1:513], data0=trs[(h, 0)][:], data1=zer3[:],
                        initial=p0s[h][:, 0:1], op0=ALU.add, op1=ALU.add)
                    nc.vector.tensor_tensor_scan(
                        cumb[:, 513:1025], data0=trs[(h, 1)][:], data1=zer3[:],
                        initial=cumb[:, 512:513], op0=ALU.add, op1=ALU.add)
                    cumbs[h] = cumb

                # ---------------- phase B (both halves, pair pos-out DMA) ----
                eng_pb = ENG["phaseb"]
                for tb in range(NTB):
                    for h in (0, 1):
                        cps = psp3.tile([P, 3], F, tag="cps")
                        nc.tensor.transpose(
                            cps[:], cumbs[h][:, tb * 128:tb * 128 + 128], ident[0:3, 0:3])
                        cpsb = work.tile([P, 3], F, tag="cpsb")
                        nc.scalar.copy(cpsb[:], cps[:])
                        O = o_tiles[(b0 + h, tb)]
                        for c in (0, 2):
                            getattr(nc, eng_pb).tensor_scalar(
                                _ap(O, c, [[3, J]]), _ap(O, c, [[3, J]]),
                                cpsb[:, c:c + 1], None, ALU.add)
                    Opair = o_pairs[(bp, tb)]
                    nc.scalar.dma_start(
                        out=bass_rust.AP(
                            tensor=o_pos[:].tensor,
                            offset=b0 * pos_elems + tb * 128 * J * 3,
                            ap=[[J * 3, P], [pos_elems, 2], [1, J * 3]]),
                        in_=_ap(Opair[:], 0, [[102, 2], [1, 96]]))
    nc.compile()
    return nc


_NC_CACHE = None


def _get_nc():
    global _NC_CACHE
    if _NC_CACHE is None:
        _NC_CACHE = build_nc()
    return _NC_CACHE


def make_posv(glb_pos, glb_vel):
    posv = np.zeros((B, T, J + 2, 3), np.float32)
    posv[:, :, :J] = glb_pos
    posv[:, : T - 1, J] = glb_vel[:, :, 0, :]
    return posv


def kernel(glb_pos, glb_rot, glb_vel, root_rotation):
    from concourse.bass_utils import run_bass_kernel_spmd

    nc = _get_nc()
    posv = make_posv(glb_pos, glb_vel)
    root = np.ascontiguousarray(root_rotation[:, :, 0, :])

    in_maps = []
    for i in range(NCORES):
        s = slice(i * BPC, (i + 1) * BPC)
        in_maps.append({
            "g_posv": np.ascontiguousarray(posv[s]),
            "g_rot": np.ascontiguousarray(glb_rot[s]),
            "g_root": np.ascontiguousarray(root[s]),
        })
    res = run_bass_kernel_spmd(nc, in_maps, list(range(NCORES)))
    pos_out = np.concatenate([r["o_pos"] for r in res.results], axis=0)
    rot_out = np.concatenate([r["o_rot"] for r in res.results], axis=0)
    return pos_out, rot_out
